# revision 12
# baseline (speedup 1.0000x reference)
"""GAT-style masked self-attention (B=4, N=4096, D=128) on 8 trn2 NeuronCores.

reference:
    scores = X @ X^T / sqrt(D)            [B, N, N]
    masked = where(adj > 0, scores, -1e12)
    attn   = softmax(masked, axis=2)
    out    = attn @ X                     [B, N, D]

Sharding: 8 cores <- (batch b, row-half h); each core handles 2048 rows
of one batch element against all 4096 keys. No collectives: every core
produces its own 2048x128 output slice.

Per-core key order is ROTATED by h*R so that this core's own rows are
always key-columns 0..R-1 of its xt: the score-matmul rhs then reuses
the same xt chunks as the lhsT (no separate xtr tensor). Softmax over
keys is permutation-invariant as long as adj columns and the xaug rows
are rotated identically (they are, on the host).

Device algorithm (per core), orientation "S^T" (keys on partitions):
  - score matmul: psS = XT[:,k128].T @ XT[:,rows blk]
  - ACT evicts PSUM with exp fused: p = exp(scale*psS - 8)  (fp16; the -8
    keeps probs inside fp16 range and cancels in the softmax ratio).
    Evictions are 1024 elems (2 key tiles x 512 rows), the PSUM-bank
    maximum with double buffering; ACT is the critical engine.
  - DVE applies the 0/1 mask per 8-key-tile super group: ptm = p * adjT.
    Mask multiplies for block i are LAGGED into phase i+1 (just before
    the AV matmuls that consume them), so the mask DMA stream has a full
    extra phase of slack and never gates the ACT pipeline. The LAST
    block's masks run eagerly in its own phase so the drain is all-AV.
  - AV matmul with the denominator fused via an appended ones-column:
      psO[rc] (+)= ptm[:, k, rc128].T @ [X_k | 1]   accumulated over k
      out = psO[:, :128] * (1 / psO[:, 128])        row-wise normalize
  - softmax shift-invariance makes a row-max pass unnecessary:
    scores*scale are bounded (~|s|<16), exp stays well inside fp32 range.
  - row blocks are software-pipelined: block i runs scores/exp while
    block i-1 runs mask+AV; AV matmuls are emitted first within each
    group so PE covers the ACT drain. The last blocks are 128 rows so
    the final (unoverlapped) AV drain is short.
  - ALL DMA on the sync HWDGE ring (the software gpsimd ring's slow
    descriptors stall the shared DMA engines at startup): xt chunks in
    first-use order, xaug (host-packed partition-major, contiguous),
    then the mask stream, with output writebacks interleaved.

PSUM budget: pools allocate per-slot at BANK granularity and accumulate
resets are bank-granular, so psS = 2 bufs x 2 banks, psO = 4 slots x 1
bank -> exactly 8 banks. The warmup matmuls borrow a psS slot.
"""

import math
import sys

sys.path.insert(0, "/opt/trn_rl_repo")

import numpy as np

B, N, D = 4, 4096, 128
R = N // 2            # rows per core
NK = N // 128         # 32 key tiles
RB = 2048             # row granularity of the host-packed mask layout
NRB = R // RB
SG = 8                # key tiles per super group (one mask DMA / mask mul)
NSG = NK // SG
SCALE = 1.0 / math.sqrt(D)
EXP_BIAS = -8.0       # exp(s*scale - 8): keeps probs in fp16 range; cancels

# row blocks (offset, size): 128-row tail so the final AV drain is short
BLOCKS = [(0, 512), (512, 512), (1024, 512), (1536, 256), (1792, 128), (1920, 128)]
# xt chunks (offset, size): block-aligned; small leading chunks so the
# first score matmuls only wait on a small DMA
XCHUNKS = [(0, 512), (512, 512), (1024, 1024), (2048, 1024), (3072, 1024)]
ACT_ELEMS = 1024      # target elements per ACT eviction (PSUM-bank limited)

CFG = dict(
    score_dt="float16",
    p_dt="float16",
    adj_dt="float16",
    ptm_bufs=2,
    psum_s_bufs=2,
    adj_bufs=8,
    pet_bufs=5,
    warm_mms=0,
)

_CACHE = {}


def _groups(bs):
    """ACT eviction groups (k_offset, n_ktiles) covering one super group."""
    out, k = [], 0
    while k < SG:
        nk = min(max(ACT_ELEMS // bs, 1), SG - k)
        out.append((k, nk))
        k += nk
    return out


def _build_nc(cfg):
    from concourse import bacc
    import concourse.mybir as mybir
    from concourse.tile import TileContext

    dt = mybir.dt
    score_dt = getattr(dt, cfg["score_dt"])
    p_dt = getattr(dt, cfg["p_dt"])
    adj_dt = getattr(dt, cfg["adj_dt"])

    nc = bacc.Bacc(None, target_bir_lowering=False)

    xt_d = nc.dram_tensor("xt", [D, N], score_dt, kind="ExternalInput")
    # host-packed partition-major: xaug_p[p, t, d] = [X | 1][t*128 + p, d]
    xaug_d = nc.dram_tensor("xaug", [128, NK, D + 1], p_dt, kind="ExternalInput")
    # 0/1 mask, host-packed as [rb, key_in_tile, key_tile, row_in_block]
    adj_d = nc.dram_tensor("adjt", [NRB, 128, NK, RB], adj_dt, kind="ExternalInput")
    o_d = nc.dram_tensor("o", [R, D], dt.float32, kind="ExternalOutput")

    def adj_src(off, bs, sg):
        rb0, r0 = off // RB, off % RB
        return adj_d[rb0, :, sg * SG:(sg + 1) * SG, r0:r0 + bs]

    with TileContext(nc) as tc:
        with (
            tc.tile_pool(name="singles", bufs=1) as singles,
            tc.tile_pool(name="ptm", bufs=cfg["ptm_bufs"]) as ptm_pool,
            tc.tile_pool(name="adj", bufs=cfg["adj_bufs"]) as adj_pool,
            tc.tile_pool(name="pe", bufs=cfg["pet_bufs"]) as pe_pool,
            tc.tile_pool(name="outs", bufs=4) as out_pool,
            tc.tile_pool(name="small", bufs=4) as small_pool,
            tc.tile_pool(name="psS", bufs=cfg["psum_s_bufs"], space="PSUM") as psS_pool,
            tc.tile_pool(name="psO", bufs=4, space="PSUM") as psO_pool,
        ):
            ebias = singles.tile([128, 1], mybir.dt.float32)
            nc.vector.memset(ebias[:], EXP_BIAS)
            # warm the exp table while the init DMAs stream in
            warm = small_pool.tile([128, 1], mybir.dt.float32, tag="warm")
            nc.vector.memset(warm[:], 0.0)
            warm2 = small_pool.tile([128, 1], mybir.dt.float32, tag="warm")
            nc.scalar.activation(
                warm2[:], warm[:], mybir.ActivationFunctionType.Exp, scale=1.0
            )

            # init DMAs, all on the sync HWDGE ring, in first-use order
            xt_sb = []
            for c, (xo, xs) in enumerate(XCHUNKS):
                t = singles.tile([D, xs], score_dt, name=f"xt_{c}")
                nc.sync.dma_start(out=t[:], in_=xt_d[:, xo:xo + xs])
                xt_sb.append((xo, xs, t))
            xaug_sb = singles.tile([128, NK, D + 1], p_dt)
            nc.sync.dma_start(out=xaug_sb[:], in_=xaug_d[:, :, :])

            # PE p-state warmup: dummy matmuls on uninitialized SBUF while
            # the first xt chunk streams in (psW freed before psS/psO alloc
            # ordering matters only for space; pools here are declared
            # upfront so psW shares the pool space with real tiles -- use a
            # dedicated small pool slot)
            if cfg["warm_mms"]:
                wsrc = small_pool.tile([128, 128], score_dt, tag="wsrc")
                nc.vector.memset(wsrc[:], 0.0)
                psW = psS_pool.tile([128, 1, 512], mybir.dt.float32, tag="psS",
                                    name="psW")
                for _ in range(cfg["warm_mms"]):
                    nc.tensor.matmul(
                        psW[:, 0, 0:128],
                        lhsT=wsrc[:, :],
                        rhs=wsrc[:, :],
                        start=True,
                        stop=True,
                    )

            def xt_slice(col, width):
                for xo, xs, t in xt_sb:
                    if xo <= col and col + width <= xo + xs:
                        return t[:, col - xo:col - xo + width]
                raise AssertionError((col, width))

            NB = len(BLOCKS)
            pet_prev = None   # list of pet tiles (one per sg) of prev block
            adj_prev = None
            ptm_eager = None  # last block's masks, built in its own phase
            bs_prev = None
            off_prev = None
            for phase in range(NB + 1):
                psO = None
                ptm = None
                last_compute = phase == NB - 1
                if phase >= 1:
                    ptm = ptm_pool.tile([128, NK, bs_prev], p_dt, tag="ptm",
                                        name=f"ptm_{phase}")
                    # one PSUM bank per rc: accumulate-start resets are
                    # bank-granular, groups must not share a bank
                    psO = [
                        psO_pool.tile(
                            [128, D + 1], mybir.dt.float32,
                            tag="psO", name=f"psO_{phase}_{rc}",
                        )
                        for rc in range(bs_prev // 128)
                    ]

                def mask_mul(sg, dst, pets, adjs, bsz):
                    k0 = sg * SG
                    nc.vector.tensor_mul(
                        dst[:, k0:k0 + SG, :],
                        pets[sg][:, :, :],
                        adjs[sg][:, :, 0:bsz],
                    )

                def av_matmuls(k, src):
                    for rc in range(bs_prev // 128):
                        nc.tensor.matmul(
                            psO[rc][:, :],
                            lhsT=src[:, k, rc * 128:(rc + 1) * 128],
                            rhs=xaug_sb[:, k, :],
                            start=(k == 0),
                            stop=(k == NK - 1),
                        )

                def normalize_store(rc, nm):
                    recip = small_pool.tile([128, 1], mybir.dt.float32,
                                            tag="recip", name=f"recip_{nm}")
                    nc.vector.reciprocal(recip[:], psO[rc][:, D:D + 1])
                    o_sb = out_pool.tile([128, D], mybir.dt.float32, tag="o",
                                         name=f"o_{nm}")
                    nc.vector.tensor_scalar_mul(o_sb[:], psO[rc][:, 0:D],
                                                recip[:])
                    r0 = off_prev + rc * 128
                    nc.sync.dma_start(out=o_d[r0:r0 + 128, :], in_=o_sb[:])

                if phase == NB:
                    # drain: masks were built eagerly last phase -> pure AV
                    for k in range(NK):
                        av_matmuls(k, ptm_eager)
                    for rc in range(bs_prev // 128):
                        normalize_store(rc, f"d_{rc}")
                    break

                off, bs = BLOCKS[phase]
                # mask stream for THIS block (consumed next phase, except
                # the last block which masks eagerly)
                adj_cur = []
                for sg in range(NSG):
                    a = adj_pool.tile([128, SG, bs], adj_dt, tag="adj",
                                      name=f"adj_{phase}_{sg}")
                    nc.sync.dma_start(out=a[:], in_=adj_src(off, bs, sg))
                    adj_cur.append(a)

                if last_compute:
                    ptm_eager = ptm_pool.tile([128, NK, bs], p_dt, tag="ptm",
                                              name="ptm_eager")

                pet_cur = []
                for sg in range(NSG):
                    if phase >= 1:
                        mask_mul(sg, ptm, pet_prev, adj_prev, bs_prev)
                    pet = pe_pool.tile([128, SG, bs], p_dt, tag="pe",
                                       name=f"pe_{phase}_{sg}")
                    pet_cur.append(pet)
                    for kb, nk in _groups(bs):
                        # AV matmuls for the previous block first: PE has
                        # work while ACT drains this group's scores.
                        if phase >= 1:
                            for j in range(nk):
                                av_matmuls(sg * SG + kb + j, ptm)
                        ps = psS_pool.tile([128, nk, bs], mybir.dt.float32,
                                           tag="psS", name=f"psS_{phase}_{sg}_{kb}")
                        for j in range(nk):
                            k = sg * SG + kb + j
                            nc.tensor.matmul(
                                ps[:, j, :],
                                lhsT=xt_slice(k * 128, 128),
                                rhs=xt_slice(off, bs),
                                start=True,
                                stop=True,
                            )
                        # evict PSUM with exp fused; mask comes later
                        nc.scalar.activation(
                            pet[:, kb:kb + nk, :],
                            ps[:, 0:nk, :],
                            mybir.ActivationFunctionType.Exp,
                            bias=ebias[:],
                            scale=SCALE,
                        )
                # eager masks for the last block AFTER all lagged masks:
                # emitted mid-loop they would block the in-order DVE queue
                # (each waits on a late activate) and stall the AV stream
                if last_compute:
                    for sg in range(NSG):
                        mask_mul(sg, ptm_eager, pet_cur, adj_cur, bs)
                if phase >= 1:
                    for rc in range(bs_prev // 128):
                        normalize_store(rc, f"{phase}_{rc}")
                pet_prev = pet_cur
                adj_prev = adj_cur
                bs_prev = bs
                off_prev = off
    nc.finalize()
    return nc


def _get_nc():
    key = tuple(sorted(CFG.items()))
    if key not in _CACHE:
        _CACHE[key] = _build_nc(CFG)
    return _CACHE[key]


def _np_dt(name):
    import ml_dtypes

    return {
        "float32": np.float32,
        "float32r": np.float32,
        "bfloat16": ml_dtypes.bfloat16,
        "float16": np.float16,
    }[name]


def make_in_maps(input, adj):
    """Host-side shard/layout prep: one input map per core."""
    input = np.asarray(input, dtype=np.float32)
    adj = np.asarray(adj)
    score_np = _np_dt(CFG["score_dt"])
    p_np = _np_dt(CFG["p_dt"])
    adj_np = _np_dt(CFG["adj_dt"])

    in_maps = []
    for core in range(8):
        b, h = core // 2, core % 2
        xb = input[b]                                    # [N, D]
        # key rotation: this core's rows become key-columns 0..R-1
        xrot = np.roll(xb, -h * R, axis=0)               # [N, D], rotated keys
        xt = np.ascontiguousarray(xrot.T).astype(score_np, copy=False)
        xaug = np.concatenate([xrot, np.ones((N, 1), np.float32)], axis=1)
        # partition-major pack: xaug_p[p, t, d] = xaug[t*128 + p, d]
        xaug_p = np.ascontiguousarray(
            xaug.reshape(NK, 128, D + 1).transpose(1, 0, 2)
        ).astype(p_np)
        s = adj[b][h * R:(h + 1) * R, :]                 # [R rows, N cols]
        s = np.roll(s, -h * R, axis=1)                   # rotate key columns
        # multiplicative 0/1 mask; adjt[rb, p, k, r] = (s[rb*RB+r, k*128+p]>0)
        adjt = np.ascontiguousarray(
            (s > 0).astype(adj_np).reshape(NRB, RB, NK, 128).transpose(0, 3, 2, 1)
        )
        in_maps.append({"xt": xt, "xaug": xaug_p, "adjt": adjt})
    return in_maps


def run_device(in_maps, trace=False, trace_cores=None):
    import concourse.bass_utils as bass_utils

    if trace:
        bass_utils.upload_artifacts = lambda tmpdir: ""  # no bucket in sandbox
    nc = _get_nc()
    return bass_utils.run_bass_kernel_spmd(
        nc, in_maps, list(range(8)), trace=trace, trace_cores=trace_cores
    )


def kernel(input, adj):
    res = run_device(make_in_maps(input, adj))
    out = np.empty((B, N, D), dtype=np.float32)
    for core in range(8):
        b, h = core // 2, core % 2
        out[b, h * R:(h + 1) * R, :] = res.results[core]["o"]
    return out
